# revision 8
# baseline (speedup 1.0000x reference)
"""Multi-head self-attention (B=4, S=4096, D=128, H=4, no scaling, no mask)
on 8 Trainium2 NeuronCores.

Sharding: 16 (batch, head) pairs over 8 cores -> core c handles batch c//2,
heads 2*(c%2) and 2*(c%2)+1. No cross-core communication.

Per-core algorithm (flash-style, scores never touch DRAM), v4:
  - query blocks of 1024; scores psum tiles hold ONE 128-key chunk x 1024
    queries ([128, 1024], 2 banks, bufs=3). One matmul per tile (f32r
    moving at 1 cyc/row, row-tiled via tile_position (32*(j%2), 0), with
    2-replicated q and pair-packed kT). Shorter per-tile emission keeps
    the psum WAR recycle (scores j+3 waits exp j) near the PE roofline.
  - PV SWAPPED: the exp'd scores pt (bf16) are the STATIONARY operand
    ([128 keys x 128 queries] chunks); vhat [128 keys, 33] is the moving
    one. av[128 queries, 8*33] accumulates over all 32 key chunks in one
    psum bank -> only 33 moving rows per (key-chunk, query-chunk) instead
    of 512 (stationary loads are free): ~4x less PE time on PV. Output
    lands in [query, dim] layout, so softmax normalization is per-
    partition scalar ops and the OUT dma is contiguous.
  - av bank opened by a dummy zero matmul (start=True over all 264 cols);
    all real PV matmuls accumulate with start=False (correct under both
    whole-granule and per-byte PSUM zeroing semantics).
  - exp split across ACT (real Exp -> bf16, 18/32) and DVE (Schraudolph
    fast-exp int16(A*s+B) bitcast to bf16, 14/32; ~3% sawtooth error,
    within the 2e-2 tolerance; denominators stay consistent because the
    ones-column sums the same approximated values). Pool/GPSIMD cannot
    access PSUM so it cannot help with the exp.
  - bk is dropped entirely (softmax invariant); bq rides the q evac
    activation; bv rides the DVE vhat bias-add.
  - normalization: DVE reciprocal of the 8 ones-columns, then 4 ACT
    (Identity, scale=rcp) + 4 DVE (tensor_scalar mult) 32-col multiplies.
  - software pipeline: scores(j) emitted; exp(j-1) issued; PV(j-3)
    issued. Projections for xt tiles 1..3 interleave at slots 5/13/21 of
    block 0 with exp pre-issue (avoids psum WAR emission deadlock).
Host gathers OUT [2, S, 32] per core into the full (B, S, D) output.
"""

import sys

for _p in ("/opt/trn_rl_repo", "/root/.axon_site/_ro/trn_rl_repo"):
    if _p not in sys.path:
        sys.path.append(_p)

import numpy as np
from collections import deque
from contextlib import ExitStack

import concourse.bass as bass
import concourse.bacc as bacc
import concourse.mybir as mybir
import concourse.tile as tile
from concourse import bass_utils

F32 = mybir.dt.float32
F32R = mybir.dt.float32r
I32 = mybir.dt.int32
I16 = mybir.dt.int16
BF16 = mybir.dt.bfloat16
AF = mybir.ActivationFunctionType
ALU = mybir.AluOpType

B, D, H, HD = 4, 128, 4, 32
NCORES = 8

# Schraudolph fast-exp in bf16 bit-space: exp(x) ~= bitcast_bf16(int16(A*x+B))
# (bf16 = top 16 bits of f32, so the fp32 constants scale by 2^-16)
LOG2E = 1.4426950408889634
SCH_A = float(np.float32(2.0**7 * LOG2E))
SCH_C = 486411.0 / 2.0**16
SCH_B = float(np.float32(127.0 * 2.0**7 - SCH_C))


def _mk_pat(n, extra_a):
    pat = ["A" if i % 2 == 0 else "D" for i in range(n)]
    for i in extra_a:
        pat[i] = "A"
    return "".join(pat)


# exp engine per chunk slot (A=ACT real exp, D=DVE Schraudolph fast-exp).
# GPSIMD/Pool cannot access PSUM, so only ACT and DVE can evacuate scores.
EXP_PAT = _mk_pat(32, (7, 23))        # ACT 18/32, DVE 14/32
EXP_PAT_B0 = _mk_pat(32, (7, 23))

_built = {}


def build_nc(S):
    """Build + compile the per-core program (identical across cores)."""
    NJ = S // 128    # 128-key chunks
    NQB = S // 1024  # 1024-query blocks per head
    NT = S // 1024   # xt DMA tiles

    nc = bacc.Bacc("TRN2", target_bir_lowering=False, debug=False)

    XT = nc.dram_tensor("XT", [128, S], F32, kind="ExternalInput").ap()
    WBLOB = nc.dram_tensor("WBLOB", [128, 518], F32, kind="ExternalInput").ap()
    OUT = nc.dram_tensor("OUT", [2, S, 32], F32, kind="ExternalOutput").ap()
    # WBLOB cols: 0:128 wq (2-replicated), 128:384 wk (2x2 strided-padded),
    # 384:386 bq, 386:452 wva, 452:518 bvb(+ones)

    with tile.TileContext(nc) as tc, ExitStack() as ctx:
        const = ctx.enter_context(tc.tile_pool(name="const", bufs=1))
        big = ctx.enter_context(tc.tile_pool(name="big", bufs=1))
        pss = ctx.enter_context(tc.tile_pool(name="pss", bufs=3, space="PSUM"))
        psav = ctx.enter_context(tc.tile_pool(name="psav", bufs=2, space="PSUM"))
        work = ctx.enter_context(tc.tile_pool(name="work", bufs=6))
        outp = ctx.enter_context(tc.tile_pool(name="outp", bufs=8))

        # ---- input DMA: weights blob, then xt in NT tiles of 1024 cols
        # Service order on the shared transfer engine: xt0, blobV, blobW,
        # xt1..3 -- tile 0's v-chunks start as soon as xt0+blobV land.
        blobW = const.tile([128, 386], F32R, tag="blobW")
        blobV = const.tile([128, 132], F32R, tag="blobV")
        xts = []
        t0 = big.tile([128, 1024], F32R, tag="xt0", name="xt0")
        nc.sync.dma_start(t0[:], XT[:, 0:1024].bitcast(F32R))
        xts.append(t0)
        nc.sync.dma_start(blobV[:], WBLOB[:, 386:518].bitcast(F32R))
        nc.sync.dma_start(blobW[:], WBLOB[:, 0:386].bitcast(F32R))
        for c in range(1, NT):
            t = big.tile([128, 1024], F32R, tag=f"xt{c}", name=f"xt{c}")
            nc.sync.dma_start(t[:], XT[:, c * 1024 : (c + 1) * 1024].bitcast(F32R))
            xts.append(t)

        # combined-head weights: output partition p = 64h + 32r + e, so one
        # 128-partition matmul projects q (or packs k) for BOTH heads at once
        wq_comb = blobW[:, 0:128]
        wk_comb = [blobW[:, 128 + 128 * r : 128 + 128 * (r + 1)] for r in range(2)]
        bq_comb = blobW[:, 384:385].bitcast(F32)
        wva = blobV[:, 0:66]
        bvb = blobV[:, 66:132].bitcast(F32)

        # persistent activations (rows 64h+32r+e)
        qt_rep = big.tile([128, S], F32R, tag="qt", name="qt")
        kt_pack = big.tile([128, (NJ // 2) * 128], F32R, tag="kt", name="kt")
        # bf16: PV runs fully in bf16 (stationary pt, moving vhat)
        vhat = big.tile([128, NJ * 66], BF16, tag="vhat")

        # bf16 zeros for the av-bank-opening dummy matmul
        zbf = const.tile([128, 264], BF16, tag="zbf")
        nc.vector.memset(zbf[:], 0.0)

        # force the exp_and_others act table (covers identity+exp) up front
        scratch = const.tile([1, 1], F32, tag="scr")
        nc.scalar.activation(scratch[:], blobV[0:1, 0:1].bitcast(F32), AF.Exp)

        # p-state warm-up: ~4.5us of dummy matmuls on zeroed SBUF while the
        # input DMA is in flight, so the real projections start at full PE
        # clock (the ramp needs 3us of contiguous busy)
        zt = const.tile([128, 512], F32, tag="zt")
        nc.vector.memset(zt[:], 0.0)
        ztr = zt.bitcast(F32R)
        zp = pss.tile([128, 1024], F32, tag="s", name="zp")
        for i in range(7):
            nc.tensor.matmul(
                zp[:, 0:512], ztr[:, 0:128], ztr[:, 0:512], start=(i == 0), stop=(i == 6)
            )

        # ---- projection emitters (psum from the pss pool) ----
        def ps_tile(name):
            return pss.tile([128, 1024], F32, tag="s", name=name)

        def v_chunk(j):
            pv = ps_tile(f"pv{j}")
            nc.tensor.matmul(
                pv[:, 0:66],
                xts[j // 8][:, (j % 8) * 128 : (j % 8) * 128 + 128],
                wva,
                start=True,
                stop=True,
            )
            nc.vector.tensor_tensor(
                out=vhat[:, j * 66 : (j + 1) * 66], in0=pv[:, 0:66], in1=bvb, op=ALU.add
            )

        def k_chunk(c):
            # pack kT for chunks 8c..8c+7, both heads: partition 64h+32(j%2)+e,
            # col 128*(j//2)+p
            pk = ps_tile(f"pk{c}")
            xg = xts[c][:].rearrange("d (j p) -> d j p", p=128)
            for r in range(2):
                nc.tensor.matmul(
                    pk[:, 0:512],
                    wk_comb[r],
                    xg[:, r:8:2, :],
                    start=(r == 0),
                    stop=(r == 1),
                )
            # k-mover on ACT so DVE keeps room for the vhat bias adds
            nc.scalar.activation(
                kt_pack[:, c * 512 : (c + 1) * 512],
                pk[:, 0:512],
                AF.Identity,
            )

        def q_tile(c):
            # q for xt tile c: both 512-col halves into one psum tile, one
            # 1024-col bias evac on ACT
            pq = ps_tile(f"pq{c}")
            for u in range(2):
                nc.tensor.matmul(
                    pq[:, 512 * u : 512 * (u + 1)],
                    wq_comb,
                    xts[c][:, 512 * u : 512 * u + 512],
                    start=(u == 0),
                    stop=(u == 1),
                )
            nc.scalar.activation(
                qt_rep[:, c * 1024 : (c + 1) * 1024],
                pq[:, 0:1024],
                AF.Identity,
                bias=bq_comb,
            )

        def proj_tile(c):
            # k/q first: their movers gate the next scores chunks, while the
            # v-chunk PE work overlaps those movers
            k_chunk(c)
            q_tile(c)
            for j in range(8 * c, 8 * c + 8):
                v_chunk(j)

        # ---- attention ----
        # Decoupled software pipeline: after scores s(j) are emitted, the
        # exp of j-1 is issued and the PV of j-3.
        last_j = NJ - 1
        pending = deque()  # entries: [ps, j, av, h, q0, exp_pt]

        def issue_exp(ent, in_b0):
            ps, j, av, h, q0, _ = ent
            eng = (EXP_PAT_B0 if in_b0 else EXP_PAT)[j]
            if eng == "A":
                ptf = work.tile([128, 1024], BF16, tag="pt", name=f"pt{h}_{q0}_{j}")
                nc.scalar.activation(ptf[:], ps[:], AF.Exp)
                pt = ptf
            else:
                pti = work.tile([128, 1024], I16, tag="pti", name=f"pt{h}_{q0}_{j}")
                nc.vector.tensor_scalar(
                    out=pti[:],
                    in0=ps[:],
                    scalar1=SCH_A,
                    scalar2=SCH_B,
                    op0=ALU.mult,
                    op1=ALU.add,
                )
                pt = pti.bitcast(BF16)
            ent[5] = pt

        def issue_pv():
            ent = pending.popleft()
            if ent[5] is None:
                issue_exp(ent, False)
            _, j, av, h, q0, pt = ent
            vs = vhat[:, j * 66 + h * 33 : j * 66 + h * 33 + 33]
            for qc in range(8):
                nc.tensor.matmul(
                    av[:, qc * 33 : qc * 33 + 33],
                    pt[:, 128 * qc : 128 * qc + 128],
                    vs,
                    start=False,
                    stop=(j == last_j and qc == 7),
                    skip_group_check=True,
                )
            if j == last_j:
                # normalize straight out of psum: per-partition reciprocal of
                # the ones-columns, then eight 32-col multiplies split over
                # ACT (Identity, scale) and DVE; single contiguous block DMA.
                rcp = outp.tile([128, 8], F32, tag="rcp", name=f"rc{h}_{q0}")
                nc.vector.reciprocal(rcp[:], av[:, 32:264:33])
                osb = outp.tile([128, 256], F32, tag="osb", name=f"ob{h}_{q0}")
                for qc in range(8):
                    if qc % 2 == 0:
                        nc.scalar.activation(
                            osb[:, qc * 32 : qc * 32 + 32],
                            av[:, qc * 33 : qc * 33 + 32],
                            AF.Identity,
                            scale=rcp[:, qc : qc + 1],
                        )
                    else:
                        nc.vector.tensor_scalar(
                            out=osb[:, qc * 32 : qc * 32 + 32],
                            in0=av[:, qc * 33 : qc * 33 + 32],
                            scalar1=rcp[:, qc : qc + 1],
                            scalar2=None,
                            op0=ALU.mult,
                        )
                nc.sync.dma_start(
                    OUT[h, q0 : q0 + 1024, :].rearrange("(c p) d -> p c d", c=8),
                    osb[:].rearrange("p (c d) -> p c d", c=8),
                )

        def on_chunk(in_b0):
            if len(pending) >= 2 and pending[-2][5] is None:
                issue_exp(pending[-2], in_b0)
            if len(pending) >= 4:
                issue_pv()

        def flush_all():
            for ent in pending:
                if ent[5] is None:
                    issue_exp(ent, False)
            while pending:
                issue_pv()

        proj_tile(0)
        for h in range(2):
            for i0 in range(NQB):
                q0 = i0 * 1024
                in_b0 = h == 0 and i0 == 0
                av = psav.tile([128, 512], F32, tag="av", name=f"av{h}_{q0}")
                # open the accumulation bank: zeros over all 264 cols
                nc.tensor.matmul(
                    av[:, 0:264],
                    zbf[:, 0:128],
                    zbf[:, 0:264],
                    start=True,
                    stop=False,
                    skip_group_check=True,
                )
                for j in range(NJ):
                    ps = pss.tile([128, 1024], F32, tag="s", name=f"s{h}_{q0}_{j}")
                    off = 64 * h + 32 * (j % 2)
                    nc.tensor.matmul(
                        ps[:, 0:1024],
                        kt_pack[off : off + 32, (j // 2) * 128 : (j // 2) * 128 + 128],
                        qt_rep[off : off + 32, q0 : q0 + 1024],
                        start=True,
                        stop=True,
                        tile_position=(off, 0),
                    )
                    pending.append([ps, j, av, h, q0, None])
                    on_chunk(in_b0)
                    # interleave remaining xt-tile projections into block 0;
                    # pre-issue pending exps so the 11 psum allocations never
                    # WAR-wait on a not-yet-emitted exp (emission deadlock) —
                    # the PV backlog itself can stay pending.
                    if in_b0 and j in (5, 13, 21):
                        c = j // 8 + 1
                        if c < NT:
                            for ent in pending:
                                if ent[5] is None:
                                    issue_exp(ent, True)
                            proj_tile(c)
        flush_all()

    nc.compile()
    return nc


def _host_prep(x, Wq, bq, Wk, bk, Wv, bv, S):
    """Per-core input maps."""
    in_maps = []
    for c in range(NCORES):
        b, hp = c // 2, c % 2
        h0, h1 = 2 * hp, 2 * hp + 1
        xt = np.ascontiguousarray(x[b].T).astype(np.float32)  # [128, S]
        blob = np.zeros((128, 518), np.float32)
        for i, hh in enumerate((h0, h1)):
            wq_h = Wq[hh * 32 : (hh + 1) * 32, :]  # [32, 128]
            wk_h = Wk[hh * 32 : (hh + 1) * 32, :]
            # combined-head layout: output partition p = 64i + 32r + e
            blob[:, 64 * i : 64 * (i + 1)] = np.tile(wq_h.T, (1, 2))
            for r in range(2):
                off = 128 + 128 * r + 64 * i + 32 * r
                blob[:, off : off + 32] = wk_h.T
            blob[64 * i : 64 * (i + 1), 384] = np.tile(bq[hh * 32 : (hh + 1) * 32], 2)
            blob[:, 386 + 33 * i : 386 + 33 * i + 32] = Wv[hh * 32 : (hh + 1) * 32, :].T
            blob[:, 452 + 33 * i : 452 + 33 * i + 32] = bv[hh * 32 : (hh + 1) * 32][None, :]
            blob[:, 452 + 33 * i + 32] = 1.0
        in_maps.append({"XT": xt, "WBLOB": blob})
    return in_maps


def _unshard(results, S):
    out = np.empty((B, S, D), np.float32)
    for c in range(NCORES):
        b, hp = c // 2, c % 2
        oc = results[c]["OUT"]  # [2, S, 32]
        for hl in range(2):
            hh = 2 * hp + hl
            out[b, :, hh * 32 : (hh + 1) * 32] = oc[hl]
    return out


def _run_once(args):
    x, Wq, bq, Wk, bk, Wv, bv = args
    S = x.shape[1]
    if S not in _built:
        _built[S] = build_nc(S)
    nc = _built[S]
    in_maps = _host_prep(x, Wq, bq, Wk, bk, Wv, bv, S)
    res = bass_utils.run_bass_kernel_spmd(nc, in_maps, core_ids=list(range(NCORES)))
    return _unshard(res.results, S)


def _subproc_entry(args):
    return _run_once(args)


def kernel(x, Wq, bq, Wk, bk, Wv, bv):
    args = tuple(
        np.asarray(a, dtype=np.float32) for a in (x, Wq, bq, Wk, bk, Wv, bv)
    )
    # The axon/NRT stack occasionally fails a first dispatch with
    # NRT_EXEC_UNIT_UNRECOVERABLE (device auto-recovers). Retry in-process,
    # then in a fresh spawned process (compile caches make that cheap).
    try:
        return _run_once(args)
    except Exception:
        try:
            return _run_once(args)
        except Exception:
            import multiprocessing as mp

            ctx = mp.get_context("spawn")
            with ctx.Pool(1) as pool:
                return pool.apply(_subproc_entry, (args,))
